# revision 20
# baseline (speedup 1.0000x reference)
"""Single-head attention (B=4, L=4096, EMB=312, HID=256) on 8 NeuronCores.

Sharding: data-parallel over batch (4) x key-parallel (2) = 8 cores. Each
core handles ALL 4096 queries against its half of the keys and returns the
UNNORMALIZED partial [sum_k p*v | sum_k p] rows; the host combines the two
halves as (o1+o2)/(s1+s2). Key-sharding (vs query-sharding) halves the
duplicated K/V projection work; only the Q projection is duplicated.

Device algorithm (per core):
  - Inputs arrive pre-transposed/padded from the host. emb and W* are fp16
    (values are small-range, so fp16's 11-bit mantissa beats bf16 and loads
    half the bytes of fp32); projections are single-pass fp16 matmuls
    accumulated in fp32 PSUM (1 cycle/row on the PE, same rate as bf16).
  - embT carries a ones-row at index EMB and W* carry the bias in that row,
    so projections fold the bias in. Wv has 2 extra columns: ones (gives the
    softmax row-sum through the P@V matmul) and zero padding (even N).
  - q/k/v are stored as fp32r; QK and PV run single-pass fp32r matmuls
    (1 cycle/row at these tile widths, ~tf32 operand precision).
  - Scores are computed transposed: sT[kl, ql] = kT-chunk^T @ qT, so the
    exp() output is directly the stationary operand for the P@V matmul —
    no on-device transposes anywhere.
  - Mask is host-side transposed and encoded as fp8e4m3 {0, -240}: adding
    -240 to a score makes exp() underflow to exactly 0.0 in fp32, which is
    indistinguishable from the reference's -1e5 (no row is fully masked).
    The DVE applies it additively to the score PSUM; exp() on ACT.
  - DMA discipline: the TimelineSim charges ~565-667ns of sequencer time
    per dma_start on the SP/ACT/DVE rings plus a shared-HWDGE hold, so
    transfers are batched: 12 emb block loads + 3 weight loads + 8 mask
    loads (one per query tile, on the gpsimd/SWDGE ring which bypasses
    HWDGE) + 8 output stores. ~31 DMAs total vs 232 in the bf16x2 version.
  - The raw partials (P@V columns + row-sum column) go back to the host,
    which normalizes after combining the key-halves.

Env overrides (debug): BASS_KERNEL_MASK_RING=gpsimd|scalar,
BASS_KERNEL_MASK_DT=f8|bf16.
"""
import os

import numpy as np
import ml_dtypes

import concourse.bacc as bacc
import concourse.tile as tile
from concourse import mybir, bass2jax
from concourse.bass_utils import run_bass_kernel_spmd

# Debug aid (opt-in): surface real compile errors from the PJRT compile
# hook, which the C++ bridge otherwise swallows.
if os.environ.get("BASS_KERNEL_DEBUG"):
    import functools as _ft
    import traceback as _tb
    _orig_hook = bass2jax.neuronx_cc_hook
    @_ft.wraps(_orig_hook)
    def _dbg_hook(*args, **kwargs):
        try:
            return _orig_hook(*args, **kwargs)
        except BaseException:
            _tb.print_exc()
            raise
    bass2jax.neuronx_cc_hook = _dbg_hook

EMB, HID, B, L = 312, 256, 4, 4096
NCORES = 8
P = 128
KL = L // 2            # key rows per core (key-parallel halves)
EPAD = 384             # emb dim padded to 3 partition chunks; row EMB is the ones-row
HV = HID + 2           # v columns: HID values | ones | zero pad (even N)
QT = 512               # ql tile width (PSUM bank = 512 fp32)
NKC = KL // P          # 16 kl chunks per core
NQT = L // QT          # 8 ql tiles per core (all queries)
NKT = KL // QT         # 4 l tiles for the k projection
MASK_VAL = np.float32(-240.0)   # exactly representable in fp8e4m3

F32 = mybir.dt.float32
F16 = mybir.dt.float16
F32R = mybir.dt.float32r
F8 = mybir.dt.float8e4
BF16 = mybir.dt.bfloat16
F16NP = np.float16
F8NP = ml_dtypes.float8_e4m3

_CACHE = {}


def _mask_cfg():
    ring = os.environ.get("BASS_KERNEL_MASK_RING", "gpsimd")
    dt = os.environ.get("BASS_KERNEL_MASK_DT", "f8")
    return ring, dt


def _build():
    mask_ring, mask_dt = _mask_cfg()
    MDT = F8 if mask_dt == "f8" else BF16

    nc = bacc.Bacc(None)

    embT = nc.dram_tensor("embT", [EPAD, L], F16, kind="ExternalInput")
    wq = nc.dram_tensor("wq", [EPAD, HID], F16, kind="ExternalInput")
    wk = nc.dram_tensor("wk", [EPAD, HID], F16, kind="ExternalInput")
    wv = nc.dram_tensor("wv", [EPAD, HV], F16, kind="ExternalInput")
    maskT = nc.dram_tensor("maskT", [KL, L], MDT, kind="ExternalInput")
    out = nc.dram_tensor("out", [L, HID + 1], F32, kind="ExternalOutput")

    with tile.TileContext(nc) as tc:
        with (
            tc.tile_pool(name="big", bufs=1) as big,
            tc.tile_pool(name="wp", bufs=1) as wp,
            tc.tile_pool(name="mt", bufs=2) as mtp,
            tc.tile_pool(name="pt", bufs=6) as ptp,
            tc.tile_pool(name="fin", bufs=2) as fin,
            tc.tile_pool(name="ps_st", bufs=4, space="PSUM") as ps_st,
            tc.tile_pool(name="ps_pv", bufs=1, space="PSUM") as ps_pv,
        ):
            # ---- input loads. Weight tensors ride the ACT ring; emb blocks
            # ride the SP ring, lowest columns first so the first projection
            # matmuls start a couple of us in. Each DMA covers all 3
            # emb-chunks of its column block (partition p reads rows
            # {p, 128+p, 256+p}).
            # PE warm-up: the tensor engine ramps 0.65 -> 1.2 -> 2.4 GHz over
            # its first ~3us of activity. A dependency-free matmul chain on
            # an (uninitialized, never-read) scratch tile starts the ramp at
            # t=0 so it completes inside the startup DMA window; the real
            # projections then run at full clock from their first cycle.
            warm_in = wp.tile([P, 2 * P], F16, name="warm_in")
            nc.gpsimd.memset(warm_in, 0.0)
            warm_ps = ps_pv.tile([P, 4, QT], F32, name="pv", tag="pv")
            NWARM = 16
            for i in range(NWARM):
                nc.tensor.matmul(
                    warm_ps[:, 0, :2 * P], lhsT=warm_in[:, :P], rhs=warm_in,
                    start=(i == 0), stop=(i == NWARM - 1),
                )

            wk_t = wp.tile([P, 3, HID], F16, name="wk_t")
            wv_t = wp.tile([P, 3, HV], F16, name="wv_t")
            wq_t = wp.tile([P, 3, HID], F16, name="wq_t")
            # wk leads on the SP ring (lowest fixed issue cost) since the
            # very first projection matmul needs it; wv/wq ride the ACT ring
            # in parallel.
            nc.sync.dma_start(
                out=wk_t, in_=wk[:, :].rearrange("(c p) n -> p c n", p=P))
            for t, d in ((wv_t, wv), (wq_t, wq)):
                nc.scalar.dma_start(
                    out=t, in_=d[:, :].rearrange("(c p) n -> p c n", p=P))

            # The host rotates each core's query columns so its key-half
            # occupies columns 0..KL-1 (undone host-side on the output), so
            # the K/V projections read a PREFIX of embT and no separate
            # embTk load is needed.
            embT_t = big.tile([P, 3, L], F16, name="embT_t")
            for b0 in range(0, L, QT):
                if b0 == 0:
                    # Split the first block per emb-chunk so the very first
                    # projection matmul (which only needs chunk 0) starts
                    # ~1us earlier.
                    for cch in range(3):
                        nc.sync.dma_start(
                            out=embT_t[:, cch, 0:QT],
                            in_=embT[cch * P:(cch + 1) * P, 0:QT],
                        )
                else:
                    nc.sync.dma_start(
                        out=embT_t[:, :, b0:b0 + QT],
                        in_=embT[:, b0:b0 + QT].rearrange("(c p) n -> p c n", p=P),
                    )

            kT_r = big.tile([P, 2, KL], F32R, name="kT_r")
            qT_r = big.tile([P, 2, L], F32R, name="qT_r")
            v_r = big.tile([P, NKC, HV], F32R, name="v_r")

            # ---- projections (single-pass fp16, fp32 PSUM accumulate).
            # q/k in [h(part), hc, l(free)] layout; v in [kl(part), klc, h].
            # k/q PSUM->SBUF copies go to the DVE and v copies to ACT so the
            # copy work never gates the PE during the projection phase.
            def emit_kq(hc, lt, which):
                ps = ps_st.tile([P, QT], F32, name="st", tag="st")
                w, dst = (wk_t, kT_r) if which == "k" else (wq_t, qT_r)
                for ei in range(3):
                    nc.tensor.matmul(
                        ps,
                        lhsT=w[:, ei, hc * P:(hc + 1) * P],
                        rhs=embT_t[:, ei, lt * QT:(lt + 1) * QT],
                        start=(ei == 0), stop=(ei == 2),
                    )
                nc.vector.tensor_copy(dst[:, hc, lt * QT:(lt + 1) * QT], ps)

            def emit_v(kc):
                ps = ps_st.tile([P, QT], F32, name="st", tag="st")
                for ei in range(3):
                    nc.tensor.matmul(
                        ps[:, :HV],
                        lhsT=embT_t[:, ei, kc * P:(kc + 1) * P],
                        rhs=wv_t[:, ei, :],
                        start=(ei == 0), stop=(ei == 2),
                    )
                nc.scalar.copy(out=v_r[:, kc, :], in_=ps[:, :HV])

            kq_tiles = [("k", hc, lt) for lt in range(NKT) for hc in range(2)]
            kq_tiles += [("q", hc, lt) for lt in range(NQT) for hc in range(2)]
            vi = 0
            for i, (which, hc, lt) in enumerate(kq_tiles):
                emit_kq(hc, lt, which)
                want_v = ((i + 1) * NKC) // len(kq_tiles)
                while vi < want_v:
                    emit_v(vi)
                    vi += 1
            while vi < NKC:
                emit_v(vi)
                vi += 1

            # ---- attention
            # Uniform lag-2 software pipeline carried ACROSS ql-tile
            # boundaries: chunk kc's P@V matmuls are emitted after chunk
            # kc+2's QK matmuls (even across ql tiles), so the PE always has
            # ~2 tiles of independent work in program order while the DVE
            # mask-add + ACT exp + pv-bank WAR release of the current chunk
            # are still in flight. One mask DMA per ql tile ([2048, 512]
            # block, gpsimd/SWDGE ring) with 3 buffers -> 2-deep prefetch.
            # pv PSUM banks are reused every tile; the staging copies (DVE)
            # are emitted at the kc==15 flush, which under lag-2 lands
            # between the next tile's mask-adds early enough that the new
            # accumulation's per-bank WAR is satisfied before the PE gets
            # there. pvs allocation for a tile happens at its kc==0 flush,
            # after those copies.
            mask_dma = nc.gpsimd if mask_ring == "gpsimd" else nc.scalar
            from collections import deque

            # pv accumulators: ONE PSUM tile [P, 4, 512] so each j block
            # owns exactly one 2KB bank (matmul outputs stay bank-local) and
            # the output staging is a single DVE copy instead of four.
            pvs_box = [None]
            LAG = 3

            def emit_pv(oqt, kc, ptile):
                if kc == 0:
                    pvs_box[0] = ps_pv.tile([P, 4, QT], F32, name="pv", tag="pv")
                pv = pvs_box[0]
                for j in range(4):
                    nc.tensor.matmul(
                        pv[:, j, :HV],
                        lhsT=ptile[:, j * P:(j + 1) * P],
                        rhs=v_r[:, kc, :],
                        start=(kc == 0), stop=(kc == NKC - 1),
                    )
                if kc == NKC - 1:
                    # Ship the unnormalized partial [sum p*v | sum p]; the
                    # host divides after combining the two key-halves. For
                    # the last ql tile the copies+stores go per-j so the
                    # kernel tail isn't serialized behind one fused copy.
                    if oqt == NQT - 1:
                        # copies split DVE/ACT, stores fan across 4 rings so
                        # nothing serializes on one sequencer at the drain.
                        rings = (nc.sync, nc.scalar, nc.gpsimd, nc.sync)
                        for j in range(4):
                            otj = fin.tile([P, HID + 1], F32, name="otj",
                                           tag=f"otj{j}")
                            if j % 2:
                                nc.scalar.copy(out=otj, in_=pv[:, j, :HID + 1])
                            else:
                                nc.vector.tensor_copy(otj, pv[:, j, :HID + 1])
                            r0 = (oqt * 4 + j) * P
                            rings[j].dma_start(out=out[r0:r0 + P, :], in_=otj)
                    else:
                        ot = fin.tile([P, 4, HID + 1], F32, name="ot", tag="ot")
                        nc.vector.tensor_copy(ot, pv[:, :, :HID + 1])
                        nc.sync.dma_start(
                            out=out[oqt * QT:(oqt + 1) * QT, :].rearrange(
                                "(j p) n -> p j n", p=P),
                            in_=ot,
                        )

            pending = deque()  # (qt, kc, p-tile) awaiting PV emission
            for qt in range(NQT):
                qsl = slice(qt * QT, (qt + 1) * QT)
                mk = mtp.tile([P, NKC, QT], MDT, name="mk", tag="mk")
                # The first two mask loads ride the SP ring, whose in-order
                # program puts them AFTER the embT blocks — otherwise the
                # Pool ring issues them at t=0 and their transfers preempt
                # the startup emb loads on the shared DMA engines. Later
                # tiles (gated by the 2-buffer pool anyway) use the Pool
                # ring, keeping the SP ring free for output stores.
                ring = nc.sync if qt < 2 else mask_dma
                ring.dma_start(
                    out=mk, in_=maskT[:, qsl].rearrange("(c p) n -> p c n", p=P))
                for kc in range(NKC):
                    st = ps_st.tile([P, QT], F32, name="st", tag="st")
                    for hc in range(2):
                        nc.tensor.matmul(
                            st,
                            lhsT=kT_r[:, hc, kc * P:(kc + 1) * P],
                            rhs=qT_r[:, hc, qsl],
                            start=(hc == 0), stop=(hc == 1),
                        )
                    if len(pending) == LAG:
                        emit_pv(*pending.popleft())
                    nc.vector.tensor_tensor(
                        out=st, in0=st, in1=mk[:, kc, :], op=mybir.AluOpType.add)
                    pt_ = ptp.tile([P, QT], F32R, name="pt", tag="pt")
                    nc.scalar.activation(
                        out=pt_, in_=st, func=mybir.ActivationFunctionType.Exp)
                    pending.append((qt, kc, pt_))
            while pending:
                emit_pv(*pending.popleft())
    nc.finalize()
    return nc


def _get_nc():
    key = "nc_turbo_" + "_".join(_mask_cfg())
    if key not in _CACHE:
        _CACHE[key] = _build()
    return _CACHE[key]


def kernel(embedding, mask, Wq, bq, Wk, bk, Wv, bv):
    embedding = np.asarray(embedding, dtype=np.float32)
    mask = np.asarray(mask, dtype=np.float32)
    Wq = np.asarray(Wq, dtype=np.float32)
    Wk = np.asarray(Wk, dtype=np.float32)
    Wv = np.asarray(Wv, dtype=np.float32)
    bq = np.asarray(bq, dtype=np.float32)
    bk = np.asarray(bk, dtype=np.float32)
    bv = np.asarray(bv, dtype=np.float32)

    _, mask_dt = _mask_cfg()
    MNP = F8NP if mask_dt == "f8" else ml_dtypes.bfloat16
    mscale = MASK_VAL if mask_dt == "f8" else np.float32(-100000.0)

    def pad_w(w, b, extra_one=False):
        wp = np.zeros((EPAD, HV if extra_one else HID), dtype=np.float32)
        wp[:EMB, :HID] = w
        wp[EMB, :HID] = b
        if extra_one:
            wp[EMB, HID] = 1.0
        return wp.astype(F16NP)

    wq_a = pad_w(Wq, bq)
    wk_a = pad_w(Wk, bk)
    wv_a = pad_w(Wv, bv, extra_one=True)

    # Each core's query columns are rotated so its key-half occupies
    # columns 0..KL-1: the device then projects K/V from a prefix of the
    # same embT tile (no separate embTk load) and the host un-rotates the
    # output rows after the gather. half=0 is the identity; half=1 swaps
    # the two halves (an involution).
    in_maps = []
    for c in range(NCORES):
        b, half = divmod(c, 2)
        embT = np.zeros((EPAD, L), dtype=np.float32)
        embT[:EMB] = embedding[b].T
        embT[EMB] = 1.0
        embT16 = embT.astype(F16NP)
        ksl = slice(half * KL, (half + 1) * KL)
        mT = (mask[b].T[ksl, :] * mscale).astype(MNP)
        if half == 1:
            embT16 = np.ascontiguousarray(
                np.concatenate([embT16[:, KL:], embT16[:, :KL]], axis=1))
            mT = np.ascontiguousarray(
                np.concatenate([mT[:, KL:], mT[:, :KL]], axis=1))
        in_maps.append({
            "embT": embT16,
            "wq": wq_a, "wk": wk_a, "wv": wv_a,
            "maskT": mT,
        })

    nc = _get_nc()
    trace = bool(int(os.environ.get("BASS_KERNEL_TRACE", "0")))
    res = run_bass_kernel_spmd(nc, in_maps, core_ids=list(range(NCORES)), trace=trace)
    _CACHE["last_results"] = res

    full = np.empty((B, L, HID), dtype=np.float32)
    for b in range(B):
        r0 = res.results[2 * b]["out"].astype(np.float64)
        r1 = res.results[2 * b + 1]["out"].astype(np.float64)
        r1 = np.concatenate([r1[KL:], r1[:KL]], axis=0)  # un-rotate half=1
        num = r0[:, :HID] + r1[:, :HID]
        den = r0[:, HID:] + r1[:, HID:]
        full[b] = (num / den).astype(np.float32)
    return full
